# revision 13
# baseline (speedup 1.0000x reference)
"""GATConv Trainium2 Bass kernel (8 NeuronCores, SPMD).

Strategy:
  - Shard edges by dst-node range: core c owns dst nodes [c*NS, (c+1)*NS).
    Segment softmax is then fully core-local (no collective on the output).
  - Node projection GEMM (f16) is data-parallel over nodes; attn_l/attn_r row
    reductions are folded into extra GEMM output columns (host builds
    W_aug = [W | Wl | Wr]).  Projected rows are packed into a 768B gather
    table row [ft f16 x256 | (dead) | el f32 x8 | pad] and AllGather'ed in
    NCH chunks (chunk-major table layout) so the collective overlaps the
    GEMM.  er stays resident in SBUF (erall) - no DRAM roundtrip.
  - Edge phase: edges sorted by dst, grouped into 128-node windows with
    per-window tile counts (max over cores).  dma_gather uses int16 indices
    (max 32767), so each window's edges are split into lo (row < 32768) /
    hi segments gathered from T[0:32768] and T[32768:], on separate swdge
    queues.  One-hot tiles oh [128e x 128n] and their transposes ohT are
    HOST-precomputed in fp8 and streamed in per chunk on the DMA engines.
    Per 128-edge tile:
      er_e = ohT.T @ er_win              (PE matmul -> [128e, 8])
      logit= el_gathered + er_e; lrelu (DVE); ee = exp(logit - C) (ACT)
      rhs  = [ft * ee_bcast | ee]  fp16  (DVE)
      acc += oh.T @ rhs                  (PE one-hot scatter matmul, PSUM)
    Per window: rst = acc[:, :256] / max(acc[:,256:264],eps) + bias -> DMA out.
"""

import math
import os
import sys

import ml_dtypes
import numpy as np

for _p in ("/opt/trn_rl_repo",):
    if _p not in sys.path:
        sys.path.insert(0, _p)

import bass_rust
import concourse.bass as bass
import concourse.mybir as mybir
from concourse.bass_utils import run_bass_kernel_spmd
from concourse.tile import TileContext
from concourse import library_config
from concourse.vector_clock import ScopedClock as _ScopedClock


def _patched_drain_and_barrier(self, tick_clock, wait_clock):
    """This walrus build caps sync waits per instruction at 2; Tile's stock
    end-of-context drain attaches the whole global clock to one Drain.
    Redistribute the excess waits onto following sync-engine nops (they all
    execute in order on SP before the barrier + sem clear)."""
    nc = self.nc
    drain_inst = nc.sync.drain()
    wait_clock.add_sem_waits(
        drain_inst.ins, _ScopedClock({None: tick_clock.global_clock})
    )
    si = drain_inst.ins.sync_info
    waits = list(si.on_wait) if (si is not None and si.on_wait) else []
    if len(waits) > 2:
        si.on_wait = waits[:2]
        for i in range(2, len(waits), 2):
            hold = nc.sync.nop(nofuse=True)
            hold.ins.sync_info = bass_rust.SyncInfo(
                on_wait=waits[i : i + 2], on_update=[]
            )
    nc.all_engine_barrier()
    assert self.sems is not None
    popped = nc._tile_sem_poison_stack.pop()
    assert popped is self._sem_poison
    nc.clear_and_free_semaphores(list(self.sems.allocated().values()))
    nc.all_engine_barrier()


TileContext._drain_and_barrier = _patched_drain_and_barrier


def _split_multi_waits(nc):
    """This walrus build encodes at most ONE sync-wait command per
    instruction; Tile emits up to two.  Split: hoist all but the last wait
    of every instruction onto single-wait NoOps on the same engine queue,
    inserted immediately before it."""
    for f in nc.m.functions:
        for b in f.blocks:
            insts = list(b.instructions)
            out = []
            changed = False
            k = 0
            for ins in insts:
                si = ins.sync_info
                if si is not None and si.on_wait is not None and len(si.on_wait) > 1:
                    waits = list(si.on_wait)
                    for w in waits[:-1]:
                        nop = mybir.InstNoOp(name=f"{ins.name}-wsplit{k}", ins=[], outs=[])
                        k += 1
                        nop.engine = ins.engine
                        nop.sync_info = bass_rust.SyncInfo(on_wait=[w], on_update=[])
                        nc.register_instruction(nop, overwrite=True)
                        out.append(nop)
                    si.on_wait = [waits[-1]]
                    changed = True
                out.append(ins)
            if changed:
                b.instructions = out

F16 = mybir.dt.float16
F32 = mybir.dt.float32
F8 = mybir.dt.float8e4
I16 = mybir.dt.int16

ALU = mybir.AluOpType
ACTF = mybir.ActivationFunctionType


class Params:
    def __init__(self, N=50000, E=800000, IN=256, H=8, O=32, NCORES=8,
                 SPLIT=32768, CHUNK_W=2, NCH=5):
        self.N, self.E, self.IN, self.H, self.O = N, E, IN, H, O
        self.HO = H * O                      # 256
        self.NCORES = NCORES
        assert N % NCORES == 0
        self.NS = N // NCORES                # nodes per core
        self.WIN = 128
        self.NWIN = math.ceil(self.NS / self.WIN)
        self.SPLIT = SPLIT                   # int16 index limit boundary
        self.CHUNK_W = CHUNK_W               # windows per gather call
        self.NCH = NCH                       # AllGather chunks
        assert self.NS % NCH == 0
        self.CH = self.NS // NCH             # rows per AG chunk per core
        # table row: 256 f16 ft | 16 f16 dead | 8 f32 el (16 f16 slots) | pad
        self.ROW = 384                       # f16 elements (768 bytes)
        self.COL_EL = self.HO + 8            # 264 (f16 slots; 8 f32)
        self.AUGC = 320                      # W_aug columns


def _wrap16(vals, npart=128):
    """Lay vals[i] at [i % 16, i // 16], replicated to npart partitions."""
    n = len(vals)
    assert n % 16 == 0
    w = vals.reshape(-1, 16).T.copy()        # [16, n//16]
    return np.tile(w, (npart // 16, 1))      # [npart, n//16]


def host_prep(p, feat, W, attn_l, attn_r, bias, src, dst):
    """Build all per-core input arrays. Returns (in_maps, meta)."""
    N, E, IN, H, O = p.N, p.E, p.IN, p.H, p.O
    feat = np.asarray(feat, np.float32)
    W = np.asarray(W, np.float32)
    attn_l = np.asarray(attn_l, np.float32)
    attn_r = np.asarray(attn_r, np.float32)
    bias = np.asarray(bias, np.float32)
    src = np.asarray(src, np.int64)
    dst = np.asarray(dst, np.int64)

    # ---- parameter preprocessing ----
    W3 = W.reshape(IN, H, O)
    Wl = np.einsum("kho,ho->kh", W3, attn_l)          # [IN, H]
    Wr = np.einsum("kho,ho->kh", W3, attn_r)          # [IN, H]
    W_aug = np.zeros((IN, p.AUGC), np.float32)
    W_aug[:, : p.HO] = W
    W_aug[:, p.HO : p.HO + H] = Wl
    W_aug[:, p.HO + H : p.HO + 2 * H] = Wr

    # softmax shift constant (upper bound on logits keeps exp in fp16 range)
    el_max = (feat @ Wl).max()
    er_max = (feat @ Wr).max()
    C = float(max(0.0, el_max + er_max - 10.0))

    bias_rep = np.tile(bias.reshape(1, p.HO), (128, 1)).astype(np.float32)

    # ---- chunk-major table row mapping ----
    # T_full row of node (c, r): k = r // CH -> k*(8*CH) + c*CH + (r % CH)
    ns_idx = np.arange(p.NS, dtype=np.int64)
    base = (ns_idx // p.CH) * (p.NCORES * p.CH) + (ns_idx % p.CH)
    rowlut = np.empty(N, np.int64)
    for c in range(p.NCORES):
        rowlut[c * p.NS : (c + 1) * p.NS] = base + c * p.CH

    # ---- edge prep ----
    order = np.argsort(dst, kind="stable")
    dst_s = dst[order]
    srow_s = rowlut[src[order]]              # table row per edge
    core_bounds = np.searchsorted(dst_s, [c * p.NS for c in range(p.NCORES + 1)])

    per_core = []  # (lo_lists, hi_lists) of (table_row, dst_in_window) per window
    for c in range(p.NCORES):
        e0, e1 = core_bounds[c], core_bounds[c + 1]
        dl = dst_s[e0:e1] - c * p.NS
        sl = srow_s[e0:e1]
        wb = np.searchsorted(dl, [w * p.WIN for w in range(p.NWIN + 1)])
        lo_lists, hi_lists = [], []
        for w in range(p.NWIN):
            a, b = wb[w], wb[w + 1]
            dw = (dl[a:b] - w * p.WIN).astype(np.int64)
            sw = sl[a:b]
            m = sw < p.SPLIT
            lo_lists.append((sw[m], dw[m]))
            hi_lists.append((sw[~m] - p.SPLIT, dw[~m]))
        per_core.append((lo_lists, hi_lists))

    # per-window tile counts: max over cores, >= 1
    TL = [max(1, max(math.ceil(len(pc[0][w][0]) / 128) for pc in per_core))
          for w in range(p.NWIN)]
    TH = [max(1, max(math.ceil(len(pc[1][w][0]) / 128) for pc in per_core))
          for w in range(p.NWIN)]
    cumT = [0]
    for w in range(p.NWIN):
        cumT.append(cumT[-1] + TL[w] + TH[w])
    NT = cumT[-1]                            # total tiles
    NLO = sum(TL)
    NHI = sum(TH)

    F8NP = ml_dtypes.float8_e4m3fn
    eye256 = np.zeros((256, 128), F8NP)
    eye256[:128] = np.eye(128).astype(F8NP)

    in_maps = []
    for c in range(p.NCORES):
        lo_lists, hi_lists = per_core[c]
        dstv = np.full((NT, 128), 255, np.int64)   # 255 = pad (no node)
        lo_cols, hi_cols = [], []
        ci = 0
        while ci < p.NWIN:
            wn = min(p.CHUNK_W, p.NWIN - ci)
            lo_idx, hi_idx = [], []
            for w in range(ci, ci + wn):
                ls, ld = lo_lists[w]
                hs, hd = hi_lists[w]
                li = np.zeros(TL[w] * 128, np.int16)
                li[: len(ls)] = ls.astype(np.int16)
                hi_ = np.zeros(TH[w] * 128, np.int16)
                hi_[: len(hs)] = hs.astype(np.int16)
                lo_idx.append(li)
                hi_idx.append(hi_)
                # dst values into global tiles: lo tiles first, then hi
                t0 = cumT[w]
                dv = dstv[t0 : t0 + TL[w] + TH[w]].reshape(-1)
                dv[: len(ld)] = ld
                dv[TL[w] * 128 : TL[w] * 128 + len(hd)] = hd
            lo_cols.append(_wrap16(np.concatenate(lo_idx)))
            hi_cols.append(_wrap16(np.concatenate(hi_idx)))
            ci += wn
        idx_lo_w = np.concatenate(lo_cols, axis=1)   # [128, NLO*8]
        idx_hi_w = np.concatenate(hi_cols, axis=1)   # [128, NHI*8]

        # host-built one-hot tiles (fp8): oh[t][e, n] = (dst(t,e) == n)
        OH = eye256[dstv]                            # [NT, 128e, 128n] f16
        oh_x = np.ascontiguousarray(
            OH.transpose(1, 0, 2).reshape(128, NT * 128))
        ohT_x = np.ascontiguousarray(
            OH.transpose(2, 0, 1).reshape(128, NT * 128))

        featT = np.zeros((IN, p.NWIN * 128), np.float16)
        featT[:, : p.NS] = feat[c * p.NS : (c + 1) * p.NS].T

        in_maps.append({
            "featT": featT,
            "W_aug": W_aug.astype(np.float16),
            "bias_rep": bias_rep,
            "idx_lo": np.ascontiguousarray(idx_lo_w),
            "idx_hi": np.ascontiguousarray(idx_hi_w),
            "oh": oh_x,
            "ohT": ohT_x,
        })

    meta = dict(TL=tuple(TL), TH=tuple(TH), NT=NT, NLO=NLO, NHI=NHI, C=C)
    return in_maps, meta


def build_nc(p, meta):
    TL, TH = meta["TL"], meta["TH"]
    NT, NLO, NHI, C = meta["NT"], meta["NLO"], meta["NHI"], meta["C"]
    NWIN, NS, ROW = p.NWIN, p.NS, p.ROW
    H, O, HO, IN = p.H, p.O, p.HO, p.IN
    TLMAX2 = max(TL[w] + (TL[w + 1] if w + 1 < NWIN else 0)
                 for w in range(0, NWIN, p.CHUNK_W))
    THMAX2 = max(TH[w] + (TH[w + 1] if w + 1 < NWIN else 0)
                 for w in range(0, NWIN, p.CHUNK_W))
    NTMAX2 = max((TL[w] + TH[w]) + (TL[w + 1] + TH[w + 1] if w + 1 < NWIN else 0)
                 for w in range(0, NWIN, p.CHUNK_W))
    cumT = [0]
    for w in range(NWIN):
        cumT.append(cumT[-1] + TL[w] + TH[w])
    rg = [list(range(p.NCORES))]

    nc = bass.Bass(num_devices=p.NCORES, num_swdge_queues=4)

    featT_x = nc.declare_dram_parameter("featT", [IN, NWIN * 128], F16, isOutput=False)
    waug_x = nc.declare_dram_parameter("W_aug", [IN, p.AUGC], F16, isOutput=False)
    bias_x = nc.declare_dram_parameter("bias_rep", [128, HO], F32, isOutput=False)
    idxlo_x = nc.declare_dram_parameter("idx_lo", [128, NLO * 8], I16, isOutput=False)
    idxhi_x = nc.declare_dram_parameter("idx_hi", [128, NHI * 8], I16, isOutput=False)
    oh_x = nc.declare_dram_parameter("oh", [128, NT * 128], F8, isOutput=False)
    ohT_x = nc.declare_dram_parameter("ohT", [128, NT * 128], F8, isOutput=False)
    out_x = nc.declare_dram_parameter("out", [NS, HO], F32, isOutput=True)

    T_slice = nc.dram_tensor("T_slice", [NS, ROW], F16)
    T_full = nc.dram_tensor("T_full", [p.N, ROW], F16, addr_space="Shared")

    n_lo_rows = min(p.SPLIT, p.N)

    with TileContext(nc) as tc:
        with (
            tc.tile_pool(name="const", bufs=1) as constp,
            tc.tile_pool(name="fT", bufs=3) as ftp,
            tc.tile_pool(name="rowsb", bufs=3) as rowp,
            tc.tile_pool(name="gemmps", bufs=2, space="PSUM") as gpsp,
            tc.tile_pool(name="glo", bufs=2) as glop,
            tc.tile_pool(name="ghi", bufs=2) as ghip,
            tc.tile_pool(name="ohc", bufs=2) as ohcp,
            tc.tile_pool(name="ohtc", bufs=2) as ohtcp,
            tc.tile_pool(name="small", bufs=8) as smp,
            tc.tile_pool(name="rhs", bufs=6) as rhsp,
            tc.tile_pool(name="res", bufs=2) as resp,
            tc.tile_pool(name="accps", bufs=2, space="PSUM") as accp,
            tc.tile_pool(name="erps", bufs=4, space="PSUM") as erpp,
        ):
            nc.gpsimd.load_library(library_config.mlp)

            # ---- resident constants ----
            wa0 = constp.tile([128, p.AUGC], F16)
            wa1 = constp.tile([128, p.AUGC], F16)
            nc.sync.dma_start(out=wa0[:, :], in_=waug_x[0:128, :])
            nc.sync.dma_start(out=wa1[:, :], in_=waug_x[128:256, :])
            bias_sb = constp.tile([128, HO], F32)
            nc.sync.dma_start(out=bias_sb[:, :], in_=bias_x[:, :])
            idxlo_sb = constp.tile([128, NLO * 8], I16)
            nc.sync.dma_start(out=idxlo_sb[:, :], in_=idxlo_x[:, :])
            idxhi_sb = constp.tile([128, NHI * 8], I16)
            nc.sync.dma_start(out=idxhi_sb[:, :], in_=idxhi_x[:, :])
            erall = constp.tile([128, NWIN * H], F16)

            # ---- GEMM phase: project node slice, build table rows ----
            # AllGather chunk k fires right after the window completing its rows.
            ag_after = {((k + 1) * p.CH - 1) // 128: k for k in range(p.NCH)}
            for nt in range(NWIN):
                rows = min(128, NS - nt * 128)
                fT0 = ftp.tile([128, 128], F16, tag="fT0")
                fT1 = ftp.tile([128, 128], F16, tag="fT1")
                nc.scalar.dma_start(out=fT0[:, :], in_=featT_x[0:128, nt * 128 : (nt + 1) * 128])
                nc.scalar.dma_start(out=fT1[:, :], in_=featT_x[128:256, nt * 128 : (nt + 1) * 128])
                ps = gpsp.tile([128, 512], F32, tag="gps", name="gps_t")[:, : p.AUGC]
                nc.tensor.matmul(ps[:, :], lhsT=fT0[:, :],
                                 rhs=wa0[:, :], start=True, stop=False)
                nc.tensor.matmul(ps[:, :], lhsT=fT1[:, :],
                                 rhs=wa1[:, :], start=False, stop=True)
                row_sb = rowp.tile([128, ROW], F16)
                # ft -> f16
                nc.vector.tensor_copy(out=row_sb[:, 0:HO], in_=ps[:, 0:HO])
                # er -> resident SBUF (erall)
                nc.vector.tensor_copy(out=erall[:, nt * H : (nt + 1) * H],
                                      in_=ps[:, HO + H : HO + 2 * H])
                # el -> f32 at COL_EL (bitcast view)
                nc.vector.tensor_copy(out=row_sb[:, p.COL_EL : p.COL_EL + 16].bitcast(F32),
                                      in_=ps[:, HO : HO + H])
                nc.sync.dma_start(out=T_slice[nt * 128 : nt * 128 + rows, :],
                                  in_=row_sb[:rows, :])
                k = ag_after.get(nt)
                if k is not None:
                    nc.gpsimd.collective_compute(
                        "AllGather", ALU.bypass, replica_groups=rg,
                        ins=[T_slice[k * p.CH : (k + 1) * p.CH, :]],
                        outs=[T_full[k * p.NCORES * p.CH : (k + 1) * p.NCORES * p.CH, :]],
                    )

            # ---- edge phase ----
            _regs = {}

            def nreg(v):
                if v not in _regs:
                    _regs[v] = nc.gpsimd.to_reg(v)
                return _regs[v]

            lo_col = 0
            hi_col = 0
            ci = 0
            while ci < NWIN:
                wn = min(p.CHUNK_W, NWIN - ci)
                nlo = sum(TL[ci : ci + wn])
                nhi = sum(TH[ci : ci + wn])
                nt_ch = nlo + nhi
                g_lo = glop.tile([128, TLMAX2, ROW], F16, tag="glo", name="glo_t")[:, :nlo, :]
                g_hi = ghip.tile([128, THMAX2, ROW], F16, tag="ghi", name="ghi_t")[:, :nhi, :]
                nc.gpsimd.dma_gather(
                    out_ap=g_lo[:, :, :], in_ap=T_full[0:n_lo_rows, :],
                    idxs_ap=idxlo_sb[:, lo_col : lo_col + nlo * 8],
                    num_idxs=nlo * 128, num_idxs_reg=nreg(nlo * 128), elem_size=ROW,
                    single_packet=False, queue_num=(ci // p.CHUNK_W) % 2)
                nc.gpsimd.dma_gather(
                    out_ap=g_hi[:, :, :], in_ap=T_full[p.SPLIT : p.N, :],
                    idxs_ap=idxhi_sb[:, hi_col : hi_col + nhi * 8],
                    num_idxs=nhi * 128, num_idxs_reg=nreg(nhi * 128), elem_size=ROW,
                    single_packet=False, queue_num=2 + (ci // p.CHUNK_W) % 2)
                lo_col += nlo * 8
                hi_col += nhi * 8
                oh_ch = ohcp.tile([128, NTMAX2, 128], F8, tag="ohc", name="ohc_t")[:, :nt_ch, :]
                ohT_ch = ohtcp.tile([128, NTMAX2, 128], F8, tag="ohtc", name="ohtc_t")[:, :nt_ch, :]
                nc.scalar.dma_start(
                    out=oh_ch[:, :, :],
                    in_=oh_x[:, cumT[ci] * 128 : cumT[ci + wn] * 128])
                nc.sync.dma_start(
                    out=ohT_ch[:, :, :],
                    in_=ohT_x[:, cumT[ci] * 128 : cumT[ci + wn] * 128])

                for wi in range(wn):
                    w = ci + wi
                    rows = min(128, NS - w * 128)
                    acc = accp.tile([128, 512], F32, tag="acc", name="acc_t")[:, : HO + H]
                    ntw = TL[w] + TH[w]
                    for t in range(ntw):
                        lo = t < TL[w]
                        if lo:
                            j = (TL[ci] if wi else 0) + t
                            grow = g_lo[:, j, :]
                        else:
                            j = (TH[ci] if wi else 0) + (t - TL[w])
                            grow = g_hi[:, j, :]
                        jc = cumT[w] - cumT[ci] + t
                        oh = oh_ch[:, jc, :]
                        ohT = ohT_ch[:, jc, :]
                        er_ps = erpp.tile([128, 512], F32, tag="erps", name="erps_t")[:, :H]
                        nc.tensor.matmul(er_ps[:, :], lhsT=ohT,
                                         rhs=erall[:, w * H : (w + 1) * H],
                                         start=True, stop=True)
                        logit = smp.tile([128, H], F32, tag="logit")
                        nc.vector.tensor_tensor(
                            out=logit[:, :],
                            in0=grow[:, p.COL_EL : p.COL_EL + 16].bitcast(F32),
                            in1=er_ps[:, :], op=ALU.add)
                        logit2 = smp.tile([128, H], F32, tag="logit2")
                        nc.vector.scalar_tensor_tensor(
                            out=logit2[:, :], in0=logit[:, :], scalar=0.2,
                            in1=logit[:, :], op0=ALU.mult, op1=ALU.max)
                        rhs_t = rhsp.tile([128, HO + H], F16)
                        nc.scalar.activation(out=rhs_t[:, HO : HO + H],
                                             in_=logit2[:, :], func=ACTF.Exp,
                                             bias=-C, scale=1.0)
                        ee32 = rhsp.tile([128, HO], F16, tag="ee32")
                        nc.scalar.activation(
                            out=ee32[:, :].rearrange("p (h o) -> p h o", h=H),
                            in_=rhs_t[:, HO : HO + H].unsqueeze(-1).broadcast_to([128, H, O]),
                            func=ACTF.Copy)
                        nc.vector.tensor_tensor(
                            out=rhs_t[:, 0:HO], in0=grow[:, 0:HO],
                            in1=ee32[:, :], op=ALU.mult)
                        nc.tensor.matmul(acc[:, :], lhsT=oh, rhs=rhs_t[:, :],
                                         start=(t == 0), stop=(t == ntw - 1))
                    # window epilogue
                    den = smp.tile([128, H], F32, tag="den")
                    nc.vector.tensor_scalar(out=den[:, :], in0=acc[:, HO : HO + H],
                                            scalar1=1e-30, scalar2=None, op0=ALU.max)
                    rec = smp.tile([128, H], F32, tag="rec")
                    nc.vector.reciprocal(out=rec[:, :], in_=den[:, :])
                    res = resp.tile([128, HO], F32, tag="res")
                    nc.vector.tensor_tensor(
                        out=res[:, :].rearrange("p (h o) -> p h o", h=H),
                        in0=acc[:, 0:HO].rearrange("p (h o) -> p h o", h=H),
                        in1=rec[:, :].unsqueeze(-1).broadcast_to([128, H, O]),
                        op=ALU.mult)
                    res2 = resp.tile([128, HO], F32, tag="res2")
                    nc.vector.tensor_tensor(out=res2[:, :], in0=res[:, :],
                                            in1=bias_sb[:, :], op=ALU.add)
                    nc.sync.dma_start(out=out_x[w * 128 : w * 128 + rows, :],
                                      in_=res2[:rows, :])
                ci += wn
    from concourse.library_overlay import lower_extended_insts

    lower_extended_insts(nc)
    _split_multi_waits(nc)
    return nc


_CACHE = {}


def kernel(feat, W, attn_l, attn_r, bias, src, dst):
    p = Params()
    in_maps, meta = host_prep(p, feat, W, attn_l, attn_r, bias, src, dst)
    key = (meta["TL"], meta["TH"], round(meta["C"], 6))
    if key not in _CACHE:
        _CACHE[key] = build_nc(p, meta)
    nc = _CACHE[key]
    res = run_bass_kernel_spmd(
        nc, in_maps, list(range(p.NCORES)),
        trace=bool(os.environ.get("BASS_TRACE")),
    )
    global LAST_EXEC_NS
    LAST_EXEC_NS = res.exec_time_ns
    out = np.concatenate([res.results[c]["out"] for c in range(p.NCORES)], axis=0)
    return out.reshape(p.N, p.H, p.O).astype(np.float32)


LAST_EXEC_NS = None


# revision 14
# speedup vs baseline: 1.2733x; 1.2733x over previous
"""GATConv Trainium2 Bass kernel (8 NeuronCores, SPMD).

Strategy:
  - Shard edges by dst-node range: core c owns dst nodes [c*NS, (c+1)*NS).
    Segment softmax is then fully core-local (no collective on the output).
  - Node projection GEMM (f16) is data-parallel over nodes; attn_l/attn_r row
    reductions are folded into extra GEMM output columns (host builds
    W_aug = [W | Wl | Wr]).  Projected rows are packed into a 768B gather
    table row [ft f16 x256 | (dead) | el f32 x8 | pad] and AllGather'ed in
    NCH chunks (chunk-major table layout) so the collective overlaps the
    GEMM.  er stays resident in SBUF (erall) - no DRAM roundtrip.
  - Edge phase: edges sorted by dst, grouped into 128-node windows with
    per-window tile counts (max over cores).  dma_gather uses int16 indices
    (max 32767), so each window's edges are split into lo (row < 32768) /
    hi segments gathered from T[0:32768] and T[32768:], on separate swdge
    queues.  One-hot tiles oh [128e x 128n] and their transposes ohT are
    HOST-precomputed in fp8 and streamed in per chunk on the DMA engines.
    Per 128-edge tile:
      er_e = ohT.T @ er_win              (PE matmul -> [128e, 8])
      logit= el_gathered + er_e; lrelu (DVE); ee = exp(logit - C) (ACT)
      rhs  = [ft * ee_bcast | ee]  fp16  (DVE)
      acc += oh.T @ rhs                  (PE one-hot scatter matmul, PSUM)
    Per window: rst = acc[:, :256] / max(acc[:,256:264],eps) + bias -> DMA out.
"""

import math
import os
import sys

import ml_dtypes
import numpy as np

for _p in ("/opt/trn_rl_repo",):
    if _p not in sys.path:
        sys.path.insert(0, _p)

import bass_rust
import concourse.bass as bass
import concourse.mybir as mybir
from concourse.bass_utils import run_bass_kernel_spmd
from concourse.tile import TileContext
from concourse import library_config
from concourse.vector_clock import ScopedClock as _ScopedClock


def _patched_drain_and_barrier(self, tick_clock, wait_clock):
    """This walrus build caps sync waits per instruction at 2; Tile's stock
    end-of-context drain attaches the whole global clock to one Drain.
    Redistribute the excess waits onto following sync-engine nops (they all
    execute in order on SP before the barrier + sem clear)."""
    nc = self.nc
    drain_inst = nc.sync.drain()
    wait_clock.add_sem_waits(
        drain_inst.ins, _ScopedClock({None: tick_clock.global_clock})
    )
    si = drain_inst.ins.sync_info
    waits = list(si.on_wait) if (si is not None and si.on_wait) else []
    if len(waits) > 2:
        si.on_wait = waits[:2]
        for i in range(2, len(waits), 2):
            hold = nc.sync.nop(nofuse=True)
            hold.ins.sync_info = bass_rust.SyncInfo(
                on_wait=waits[i : i + 2], on_update=[]
            )
    nc.all_engine_barrier()
    assert self.sems is not None
    popped = nc._tile_sem_poison_stack.pop()
    assert popped is self._sem_poison
    nc.clear_and_free_semaphores(list(self.sems.allocated().values()))
    nc.all_engine_barrier()


TileContext._drain_and_barrier = _patched_drain_and_barrier


def _split_multi_waits(nc):
    """This walrus build encodes at most ONE sync-wait command per
    instruction; Tile emits up to two.  Split: hoist all but the last wait
    of every instruction onto single-wait NoOps on the same engine queue,
    inserted immediately before it."""
    for f in nc.m.functions:
        for b in f.blocks:
            insts = list(b.instructions)
            out = []
            changed = False
            k = 0
            for ins in insts:
                si = ins.sync_info
                if si is not None and si.on_wait is not None and len(si.on_wait) > 1:
                    waits = list(si.on_wait)
                    for w in waits[:-1]:
                        nop = mybir.InstNoOp(name=f"{ins.name}-wsplit{k}", ins=[], outs=[])
                        k += 1
                        nop.engine = ins.engine
                        nop.sync_info = bass_rust.SyncInfo(on_wait=[w], on_update=[])
                        nc.register_instruction(nop, overwrite=True)
                        out.append(nop)
                    si.on_wait = [waits[-1]]
                    changed = True
                out.append(ins)
            if changed:
                b.instructions = out

F16 = mybir.dt.float16
F32 = mybir.dt.float32
F8 = mybir.dt.float8e4
I16 = mybir.dt.int16

ALU = mybir.AluOpType
ACTF = mybir.ActivationFunctionType


class Params:
    def __init__(self, N=50000, E=800000, IN=256, H=8, O=32, NCORES=8,
                 SPLIT=32768, CHUNK_W=2, NCH=5):
        self.N, self.E, self.IN, self.H, self.O = N, E, IN, H, O
        self.HO = H * O                      # 256
        self.NCORES = NCORES
        assert N % NCORES == 0
        self.NS = N // NCORES                # nodes per core
        self.WIN = 128
        self.NWIN = math.ceil(self.NS / self.WIN)
        self.SPLIT = SPLIT                   # int16 index limit boundary
        self.CHUNK_W = CHUNK_W               # windows per gather call
        self.NCH = NCH                       # AllGather chunks
        assert self.NS % NCH == 0
        self.CH = self.NS // NCH             # rows per AG chunk per core
        # table row: 256 f16 ft | 16 f16 dead | 8 f32 el (16 f16 slots) | pad
        self.ROW = 384                       # f16 elements (768 bytes)
        self.COL_EL = self.HO + 8            # 264 (f16 slots; 8 f32)
        self.AUGC = 320                      # W_aug columns


def _wrap16(vals, npart=128):
    """Lay vals[i] at [i % 16, i // 16], replicated to npart partitions."""
    n = len(vals)
    assert n % 16 == 0
    w = vals.reshape(-1, 16).T.copy()        # [16, n//16]
    return np.tile(w, (npart // 16, 1))      # [npart, n//16]


def host_prep(p, feat, W, attn_l, attn_r, bias, src, dst):
    """Build all per-core input arrays. Returns (in_maps, meta)."""
    N, E, IN, H, O = p.N, p.E, p.IN, p.H, p.O
    feat = np.asarray(feat, np.float32)
    W = np.asarray(W, np.float32)
    attn_l = np.asarray(attn_l, np.float32)
    attn_r = np.asarray(attn_r, np.float32)
    bias = np.asarray(bias, np.float32)
    src = np.asarray(src, np.int64)
    dst = np.asarray(dst, np.int64)

    # ---- parameter preprocessing ----
    W3 = W.reshape(IN, H, O)
    Wl = np.einsum("kho,ho->kh", W3, attn_l)          # [IN, H]
    Wr = np.einsum("kho,ho->kh", W3, attn_r)          # [IN, H]
    W_aug = np.zeros((IN, p.AUGC), np.float32)
    W_aug[:, : p.HO] = W
    W_aug[:, p.HO : p.HO + H] = Wl
    W_aug[:, p.HO + H : p.HO + 2 * H] = Wr

    # softmax shift constant (upper bound on logits keeps exp in fp16 range)
    el_max = (feat @ Wl).max()
    er_max = (feat @ Wr).max()
    C = float(max(0.0, el_max + er_max - 10.0))

    bias_rep = np.tile(bias.reshape(1, p.HO), (128, 1)).astype(np.float32)

    # ---- chunk-major table row mapping ----
    # T_full row of node (c, r): k = r // CH -> k*(8*CH) + c*CH + (r % CH)
    ns_idx = np.arange(p.NS, dtype=np.int64)
    base = (ns_idx // p.CH) * (p.NCORES * p.CH) + (ns_idx % p.CH)
    rowlut = np.empty(N, np.int64)
    for c in range(p.NCORES):
        rowlut[c * p.NS : (c + 1) * p.NS] = base + c * p.CH

    # ---- edge prep ----
    order = np.argsort(dst, kind="stable")
    dst_s = dst[order]
    srow_s = rowlut[src[order]]              # table row per edge
    core_bounds = np.searchsorted(dst_s, [c * p.NS for c in range(p.NCORES + 1)])

    per_core = []  # (lo_lists, hi_lists) of (table_row, dst_in_window) per window
    for c in range(p.NCORES):
        e0, e1 = core_bounds[c], core_bounds[c + 1]
        dl = dst_s[e0:e1] - c * p.NS
        sl = srow_s[e0:e1]
        wb = np.searchsorted(dl, [w * p.WIN for w in range(p.NWIN + 1)])
        lo_lists, hi_lists = [], []
        for w in range(p.NWIN):
            a, b = wb[w], wb[w + 1]
            dw = (dl[a:b] - w * p.WIN).astype(np.int64)
            sw = sl[a:b]
            m = sw < p.SPLIT
            lo_lists.append((sw[m], dw[m]))
            hi_lists.append((sw[~m] - p.SPLIT, dw[~m]))
        per_core.append((lo_lists, hi_lists))

    # per-window tile counts: max over cores, >= 1
    TL = [max(1, max(math.ceil(len(pc[0][w][0]) / 128) for pc in per_core))
          for w in range(p.NWIN)]
    TH = [max(1, max(math.ceil(len(pc[1][w][0]) / 128) for pc in per_core))
          for w in range(p.NWIN)]
    cumT = [0]
    for w in range(p.NWIN):
        cumT.append(cumT[-1] + TL[w] + TH[w])
    NT = cumT[-1]                            # total tiles
    NLO = sum(TL)
    NHI = sum(TH)

    F8NP = ml_dtypes.float8_e4m3fn
    eye256 = np.zeros((256, 128), F8NP)
    eye256[:128] = np.eye(128).astype(F8NP)

    in_maps = []
    for c in range(p.NCORES):
        lo_lists, hi_lists = per_core[c]
        dstv = np.full((NT, 128), 255, np.int64)   # 255 = pad (no node)
        lo_cols, hi_cols = [], []
        ci = 0
        while ci < p.NWIN:
            wn = min(p.CHUNK_W, p.NWIN - ci)
            lo_idx, hi_idx = [], []
            for w in range(ci, ci + wn):
                ls, ld = lo_lists[w]
                hs, hd = hi_lists[w]
                li = np.zeros(TL[w] * 128, np.int16)
                li[: len(ls)] = ls.astype(np.int16)
                hi_ = np.zeros(TH[w] * 128, np.int16)
                hi_[: len(hs)] = hs.astype(np.int16)
                lo_idx.append(li)
                hi_idx.append(hi_)
                # dst values into global tiles: lo tiles first, then hi
                t0 = cumT[w]
                dv = dstv[t0 : t0 + TL[w] + TH[w]].reshape(-1)
                dv[: len(ld)] = ld
                dv[TL[w] * 128 : TL[w] * 128 + len(hd)] = hd
            lo_cols.append(_wrap16(np.concatenate(lo_idx)))
            hi_cols.append(_wrap16(np.concatenate(hi_idx)))
            ci += wn
        idx_lo_w = np.concatenate(lo_cols, axis=1)   # [128, NLO*8]
        idx_hi_w = np.concatenate(hi_cols, axis=1)   # [128, NHI*8]

        # host-built one-hot tiles (fp8): oh[t][e, n] = (dst(t,e) == n)
        OH = eye256[dstv]                            # [NT, 128e, 128n] f16
        oh_x = np.ascontiguousarray(
            OH.transpose(1, 0, 2).reshape(128, NT * 128))
        ohT_x = np.ascontiguousarray(
            OH.transpose(2, 0, 1).reshape(128, NT * 128))

        featT = np.zeros((IN, p.NWIN * 128), np.float16)
        featT[:, : p.NS] = feat[c * p.NS : (c + 1) * p.NS].T

        in_maps.append({
            "featT": featT,
            "W_aug": W_aug.astype(np.float16),
            "bias_rep": bias_rep,
            "idx_lo": np.ascontiguousarray(idx_lo_w),
            "idx_hi": np.ascontiguousarray(idx_hi_w),
            "oh": oh_x,
            "ohT": ohT_x,
        })

    meta = dict(TL=tuple(TL), TH=tuple(TH), NT=NT, NLO=NLO, NHI=NHI, C=C)
    return in_maps, meta


def build_nc(p, meta):
    TL, TH = meta["TL"], meta["TH"]
    NT, NLO, NHI, C = meta["NT"], meta["NLO"], meta["NHI"], meta["C"]
    NWIN, NS, ROW = p.NWIN, p.NS, p.ROW
    H, O, HO, IN = p.H, p.O, p.HO, p.IN
    TLMAX2 = max(TL[w] + (TL[w + 1] if w + 1 < NWIN else 0)
                 for w in range(0, NWIN, p.CHUNK_W))
    THMAX2 = max(TH[w] + (TH[w + 1] if w + 1 < NWIN else 0)
                 for w in range(0, NWIN, p.CHUNK_W))
    NTMAX2 = max((TL[w] + TH[w]) + (TL[w + 1] + TH[w + 1] if w + 1 < NWIN else 0)
                 for w in range(0, NWIN, p.CHUNK_W))
    cumT = [0]
    for w in range(NWIN):
        cumT.append(cumT[-1] + TL[w] + TH[w])
    rg = [list(range(p.NCORES))]

    nc = bass.Bass(num_devices=p.NCORES, num_swdge_queues=4)

    featT_x = nc.declare_dram_parameter("featT", [IN, NWIN * 128], F16, isOutput=False)
    waug_x = nc.declare_dram_parameter("W_aug", [IN, p.AUGC], F16, isOutput=False)
    bias_x = nc.declare_dram_parameter("bias_rep", [128, HO], F32, isOutput=False)
    idxlo_x = nc.declare_dram_parameter("idx_lo", [128, NLO * 8], I16, isOutput=False)
    idxhi_x = nc.declare_dram_parameter("idx_hi", [128, NHI * 8], I16, isOutput=False)
    oh_x = nc.declare_dram_parameter("oh", [128, NT * 128], F8, isOutput=False)
    ohT_x = nc.declare_dram_parameter("ohT", [128, NT * 128], F8, isOutput=False)
    out_x = nc.declare_dram_parameter("out", [NS, HO], F32, isOutput=True)

    T_slice = nc.dram_tensor("T_slice", [NS, ROW], F16)
    T_full = nc.dram_tensor("T_full", [p.N, ROW], F16, addr_space="Shared")

    n_lo_rows = min(p.SPLIT, p.N)

    with TileContext(nc) as tc:
        with (
            tc.tile_pool(name="const", bufs=1) as constp,
            tc.tile_pool(name="fT", bufs=3) as ftp,
            tc.tile_pool(name="rowsb", bufs=3) as rowp,
            tc.tile_pool(name="gemmps", bufs=2, space="PSUM") as gpsp,
            tc.tile_pool(name="glo", bufs=2) as glop,
            tc.tile_pool(name="ghi", bufs=2) as ghip,
            tc.tile_pool(name="ohc", bufs=2) as ohcp,
            tc.tile_pool(name="ohtc", bufs=2) as ohtcp,
            tc.tile_pool(name="small", bufs=8) as smp,
            tc.tile_pool(name="rhs", bufs=6) as rhsp,
            tc.tile_pool(name="res", bufs=2) as resp,
            tc.tile_pool(name="accps", bufs=2, space="PSUM") as accp,
            tc.tile_pool(name="erps", bufs=4, space="PSUM") as erpp,
        ):
            nc.gpsimd.load_library(library_config.mlp)

            # ---- resident constants ----
            wa0 = constp.tile([128, p.AUGC], F16)
            wa1 = constp.tile([128, p.AUGC], F16)
            nc.sync.dma_start(out=wa0[:, :], in_=waug_x[0:128, :])
            nc.sync.dma_start(out=wa1[:, :], in_=waug_x[128:256, :])
            bias_sb = constp.tile([128, HO], F32)
            nc.sync.dma_start(out=bias_sb[:, :], in_=bias_x[:, :])
            idxlo_sb = constp.tile([128, NLO * 8], I16)
            nc.sync.dma_start(out=idxlo_sb[:, :], in_=idxlo_x[:, :])
            idxhi_sb = constp.tile([128, NHI * 8], I16)
            nc.sync.dma_start(out=idxhi_sb[:, :], in_=idxhi_x[:, :])
            erall = constp.tile([128, NWIN * H], F16)

            # ---- GEMM phase: project node slice, build table rows ----
            # AllGather chunk k fires right after the window completing its rows.
            ag_after = {((k + 1) * p.CH - 1) // 128: k for k in range(p.NCH)}
            for nt in range(NWIN):
                rows = min(128, NS - nt * 128)
                fT0 = ftp.tile([128, 128], F16, tag="fT0")
                fT1 = ftp.tile([128, 128], F16, tag="fT1")
                nc.scalar.dma_start(out=fT0[:, :], in_=featT_x[0:128, nt * 128 : (nt + 1) * 128])
                nc.scalar.dma_start(out=fT1[:, :], in_=featT_x[128:256, nt * 128 : (nt + 1) * 128])
                ps = gpsp.tile([128, 512], F32, tag="gps", name="gps_t")[:, : p.AUGC]
                nc.tensor.matmul(ps[:, :], lhsT=fT0[:, :],
                                 rhs=wa0[:, :], start=True, stop=False)
                nc.tensor.matmul(ps[:, :], lhsT=fT1[:, :],
                                 rhs=wa1[:, :], start=False, stop=True)
                row_sb = rowp.tile([128, ROW], F16)
                # ft -> f16
                nc.vector.tensor_copy(out=row_sb[:, 0:HO], in_=ps[:, 0:HO])
                # er -> resident SBUF (erall)
                nc.vector.tensor_copy(out=erall[:, nt * H : (nt + 1) * H],
                                      in_=ps[:, HO + H : HO + 2 * H])
                # el -> f32 at COL_EL (bitcast view)
                nc.vector.tensor_copy(out=row_sb[:, p.COL_EL : p.COL_EL + 16].bitcast(F32),
                                      in_=ps[:, HO : HO + H])
                nc.sync.dma_start(out=T_slice[nt * 128 : nt * 128 + rows, :],
                                  in_=row_sb[:rows, :])
                k = ag_after.get(nt)
                if k is not None:
                    nc.gpsimd.collective_compute(
                        "AllGather", ALU.bypass, replica_groups=rg,
                        ins=[T_slice[k * p.CH : (k + 1) * p.CH, :]],
                        outs=[T_full[k * p.NCORES * p.CH : (k + 1) * p.NCORES * p.CH, :]],
                    )

            # ---- edge phase ----
            _regs = {}

            def nreg(v):
                if v not in _regs:
                    _regs[v] = nc.gpsimd.to_reg(v)
                return _regs[v]

            lo_col = 0
            hi_col = 0
            ci = 0
            while ci < NWIN:
                wn = min(p.CHUNK_W, NWIN - ci)
                nlo = sum(TL[ci : ci + wn])
                nhi = sum(TH[ci : ci + wn])
                nt_ch = nlo + nhi
                g_lo = glop.tile([128, TLMAX2, ROW], F16, tag="glo", name="glo_t")[:, :nlo, :]
                g_hi = ghip.tile([128, THMAX2, ROW], F16, tag="ghi", name="ghi_t")[:, :nhi, :]
                nc.gpsimd.dma_gather(
                    out_ap=g_lo[:, :, :], in_ap=T_full[0:n_lo_rows, :],
                    idxs_ap=idxlo_sb[:, lo_col : lo_col + nlo * 8],
                    num_idxs=nlo * 128, num_idxs_reg=nreg(nlo * 128), elem_size=ROW,
                    single_packet=False, queue_num=(ci // p.CHUNK_W) % 2)
                nc.gpsimd.dma_gather(
                    out_ap=g_hi[:, :, :], in_ap=T_full[p.SPLIT : p.N, :],
                    idxs_ap=idxhi_sb[:, hi_col : hi_col + nhi * 8],
                    num_idxs=nhi * 128, num_idxs_reg=nreg(nhi * 128), elem_size=ROW,
                    single_packet=False, queue_num=2 + (ci // p.CHUNK_W) % 2)
                lo_col += nlo * 8
                hi_col += nhi * 8
                oh_ch = ohcp.tile([128, NTMAX2, 128], F8, tag="ohc", name="ohc_t")[:, :nt_ch, :]
                ohT_ch = ohtcp.tile([128, NTMAX2, 128], F8, tag="ohtc", name="ohtc_t")[:, :nt_ch, :]
                nc.scalar.dma_start(
                    out=oh_ch[:, :, :],
                    in_=oh_x[:, cumT[ci] * 128 : cumT[ci + wn] * 128])
                nc.sync.dma_start(
                    out=ohT_ch[:, :, :],
                    in_=ohT_x[:, cumT[ci] * 128 : cumT[ci + wn] * 128])

                for wi in range(wn):
                    w = ci + wi
                    rows = min(128, NS - w * 128)
                    acc = accp.tile([128, 512], F32, tag="acc", name="acc_t")[:, : HO + H]
                    ntw = TL[w] + TH[w]
                    for t in range(ntw):
                        lo = t < TL[w]
                        if lo:
                            j = (TL[ci] if wi else 0) + t
                            grow = g_lo[:, j, :]
                        else:
                            j = (TH[ci] if wi else 0) + (t - TL[w])
                            grow = g_hi[:, j, :]
                        jc = cumT[w] - cumT[ci] + t
                        oh = oh_ch[:, jc, :]
                        ohT = ohT_ch[:, jc, :]
                        er_ps = erpp.tile([128, 512], F32, tag="erps", name="erps_t")[:, :H]
                        nc.tensor.matmul(er_ps[:, :], lhsT=ohT,
                                         rhs=erall[:, w * H : (w + 1) * H],
                                         start=True, stop=True)
                        logit = smp.tile([128, H], F32, tag="logit")
                        nc.vector.tensor_tensor(
                            out=logit[:, :],
                            in0=grow[:, p.COL_EL : p.COL_EL + 16].bitcast(F32),
                            in1=er_ps[:, :], op=ALU.add)
                        logit2 = smp.tile([128, H], F32, tag="logit2")
                        nc.vector.scalar_tensor_tensor(
                            out=logit2[:, :], in0=logit[:, :], scalar=0.2,
                            in1=logit[:, :], op0=ALU.mult, op1=ALU.max)
                        rhs_t = rhsp.tile([128, HO + H], F16)
                        nc.scalar.activation(out=rhs_t[:, HO : HO + H],
                                             in_=logit2[:, :], func=ACTF.Exp,
                                             bias=-C, scale=1.0)
                        nc.vector.tensor_tensor(
                            out=rhs_t[:, 0:HO].rearrange("p (h o) -> p h o", h=H),
                            in0=grow[:, 0:HO].rearrange("p (h o) -> p h o", h=H),
                            in1=rhs_t[:, HO : HO + H].unsqueeze(-1).broadcast_to([128, H, O]),
                            op=ALU.mult)
                        nc.tensor.matmul(acc[:, :], lhsT=oh, rhs=rhs_t[:, :],
                                         start=(t == 0), stop=(t == ntw - 1))
                    # window epilogue
                    den = smp.tile([128, H], F32, tag="den")
                    nc.vector.tensor_scalar(out=den[:, :], in0=acc[:, HO : HO + H],
                                            scalar1=1e-30, scalar2=None, op0=ALU.max)
                    rec = smp.tile([128, H], F32, tag="rec")
                    nc.vector.reciprocal(out=rec[:, :], in_=den[:, :])
                    res = resp.tile([128, HO], F32, tag="res")
                    nc.vector.tensor_tensor(
                        out=res[:, :].rearrange("p (h o) -> p h o", h=H),
                        in0=acc[:, 0:HO].rearrange("p (h o) -> p h o", h=H),
                        in1=rec[:, :].unsqueeze(-1).broadcast_to([128, H, O]),
                        op=ALU.mult)
                    res2 = resp.tile([128, HO], F32, tag="res2")
                    nc.vector.tensor_tensor(out=res2[:, :], in0=res[:, :],
                                            in1=bias_sb[:, :], op=ALU.add)
                    nc.sync.dma_start(out=out_x[w * 128 : w * 128 + rows, :],
                                      in_=res2[:rows, :])
                ci += wn
    from concourse.library_overlay import lower_extended_insts

    lower_extended_insts(nc)
    _split_multi_waits(nc)
    return nc


_CACHE = {}


def kernel(feat, W, attn_l, attn_r, bias, src, dst):
    p = Params()
    in_maps, meta = host_prep(p, feat, W, attn_l, attn_r, bias, src, dst)
    key = (meta["TL"], meta["TH"], round(meta["C"], 6))
    if key not in _CACHE:
        _CACHE[key] = build_nc(p, meta)
    nc = _CACHE[key]
    res = run_bass_kernel_spmd(
        nc, in_maps, list(range(p.NCORES)),
        trace=bool(os.environ.get("BASS_TRACE")),
    )
    global LAST_EXEC_NS
    LAST_EXEC_NS = res.exec_time_ns
    out = np.concatenate([res.results[c]["out"] for c in range(p.NCORES)], axis=0)
    return out.reshape(p.N, p.H, p.O).astype(np.float32)


LAST_EXEC_NS = None


# revision 15
# speedup vs baseline: 1.3080x; 1.0273x over previous
"""GATConv Trainium2 Bass kernel (8 NeuronCores, SPMD).

Strategy:
  - Shard edges by dst-node range: core c owns dst nodes [c*NS, (c+1)*NS).
    Segment softmax is then fully core-local (no collective on the output).
  - Node projection GEMM (f16) is data-parallel over nodes; attn_l/attn_r row
    reductions are folded into extra GEMM output columns (host builds
    W_aug = [W | Wl | Wr]).  Projected rows are packed into a 768B gather
    table row [ft f16 x256 | (dead) | el f32 x8 | pad] and AllGather'ed in
    NCH chunks (chunk-major table layout) so the collective overlaps the
    GEMM.  er stays resident in SBUF (erall) - no DRAM roundtrip.
  - Edge phase: edges sorted by dst, grouped into 128-node windows with
    per-window tile counts (max over cores).  dma_gather uses int16 indices
    (max 32767), so each window's edges are split into lo (row < 32768) /
    hi segments gathered from T[0:32768] and T[32768:], on separate swdge
    queues.  One-hot tiles oh [128e x 128n] and their transposes ohT are
    HOST-precomputed in fp8 and streamed in per chunk on the DMA engines.
    Per 128-edge tile:
      er_e = ohT.T @ er_win              (PE matmul -> [128e, 8])
      logit= el_gathered + er_e; lrelu (DVE); ee = exp(logit - C) (ACT)
      rhs  = [ft * ee_bcast | ee]  fp16  (DVE)
      acc += oh.T @ rhs                  (PE one-hot scatter matmul, PSUM)
    Per window: rst = acc[:, :256] / max(acc[:,256:264],eps) + bias -> DMA out.
"""

import math
import os
import sys

import ml_dtypes
import numpy as np

for _p in ("/opt/trn_rl_repo",):
    if _p not in sys.path:
        sys.path.insert(0, _p)

import bass_rust
import concourse.bass as bass
import concourse.mybir as mybir
from concourse.bass_utils import run_bass_kernel_spmd
from concourse.tile import TileContext
from concourse import library_config
from concourse.vector_clock import ScopedClock as _ScopedClock


def _patched_drain_and_barrier(self, tick_clock, wait_clock):
    """This walrus build caps sync waits per instruction at 2; Tile's stock
    end-of-context drain attaches the whole global clock to one Drain.
    Redistribute the excess waits onto following sync-engine nops (they all
    execute in order on SP before the barrier + sem clear)."""
    nc = self.nc
    drain_inst = nc.sync.drain()
    wait_clock.add_sem_waits(
        drain_inst.ins, _ScopedClock({None: tick_clock.global_clock})
    )
    si = drain_inst.ins.sync_info
    waits = list(si.on_wait) if (si is not None and si.on_wait) else []
    if len(waits) > 2:
        si.on_wait = waits[:2]
        for i in range(2, len(waits), 2):
            hold = nc.sync.nop(nofuse=True)
            hold.ins.sync_info = bass_rust.SyncInfo(
                on_wait=waits[i : i + 2], on_update=[]
            )
    nc.all_engine_barrier()
    assert self.sems is not None
    popped = nc._tile_sem_poison_stack.pop()
    assert popped is self._sem_poison
    nc.clear_and_free_semaphores(list(self.sems.allocated().values()))
    nc.all_engine_barrier()


TileContext._drain_and_barrier = _patched_drain_and_barrier


def _split_multi_waits(nc):
    """This walrus build encodes at most ONE sync-wait command per
    instruction; Tile emits up to two.  Split: hoist all but the last wait
    of every instruction onto single-wait NoOps on the same engine queue,
    inserted immediately before it."""
    for f in nc.m.functions:
        for b in f.blocks:
            insts = list(b.instructions)
            out = []
            changed = False
            k = 0
            for ins in insts:
                si = ins.sync_info
                if si is not None and si.on_wait is not None and len(si.on_wait) > 1:
                    waits = list(si.on_wait)
                    for w in waits[:-1]:
                        nop = mybir.InstNoOp(name=f"{ins.name}-wsplit{k}", ins=[], outs=[])
                        k += 1
                        nop.engine = ins.engine
                        nop.sync_info = bass_rust.SyncInfo(on_wait=[w], on_update=[])
                        nc.register_instruction(nop, overwrite=True)
                        out.append(nop)
                    si.on_wait = [waits[-1]]
                    changed = True
                out.append(ins)
            if changed:
                b.instructions = out

F16 = mybir.dt.float16
F32 = mybir.dt.float32
F8 = mybir.dt.float8e4
I16 = mybir.dt.int16

ALU = mybir.AluOpType
ACTF = mybir.ActivationFunctionType


class Params:
    def __init__(self, N=50000, E=800000, IN=256, H=8, O=32, NCORES=8,
                 SPLIT=32768, CHUNK_W=2, NCH=5):
        self.N, self.E, self.IN, self.H, self.O = N, E, IN, H, O
        self.HO = H * O                      # 256
        self.NCORES = NCORES
        assert N % NCORES == 0
        self.NS = N // NCORES                # nodes per core
        self.WIN = 128
        self.NWIN = math.ceil(self.NS / self.WIN)
        self.SPLIT = SPLIT                   # int16 index limit boundary
        self.CHUNK_W = CHUNK_W               # windows per gather call
        self.NCH = NCH                       # AllGather chunks
        assert self.NS % NCH == 0
        self.CH = self.NS // NCH             # rows per AG chunk per core
        # table row: 256 f16 ft | 16 f16 dead | 8 f32 el (16 f16 slots) | pad
        self.ROW = 384                       # f16 elements (768 bytes)
        self.COL_EL = self.HO + 8            # 264 (f16 slots; 8 f32)
        self.AUGC = 320                      # W_aug columns


def _wrap16(vals, npart=128):
    """Lay vals[i] at [i % 16, i // 16], replicated to npart partitions."""
    n = len(vals)
    assert n % 16 == 0
    w = vals.reshape(-1, 16).T.copy()        # [16, n//16]
    return np.tile(w, (npart // 16, 1))      # [npart, n//16]


def host_prep(p, feat, W, attn_l, attn_r, bias, src, dst):
    """Build all per-core input arrays. Returns (in_maps, meta)."""
    N, E, IN, H, O = p.N, p.E, p.IN, p.H, p.O
    feat = np.asarray(feat, np.float32)
    W = np.asarray(W, np.float32)
    attn_l = np.asarray(attn_l, np.float32)
    attn_r = np.asarray(attn_r, np.float32)
    bias = np.asarray(bias, np.float32)
    src = np.asarray(src, np.int64)
    dst = np.asarray(dst, np.int64)

    # ---- parameter preprocessing ----
    W3 = W.reshape(IN, H, O)
    Wl = np.einsum("kho,ho->kh", W3, attn_l)          # [IN, H]
    Wr = np.einsum("kho,ho->kh", W3, attn_r)          # [IN, H]
    W_aug = np.zeros((IN, p.AUGC), np.float32)
    W_aug[:, : p.HO] = W
    W_aug[:, p.HO : p.HO + H] = Wl
    W_aug[:, p.HO + H : p.HO + 2 * H] = Wr

    # softmax shift constant (upper bound on logits keeps exp in fp16 range)
    el_max = (feat @ Wl).max()
    er_max = (feat @ Wr).max()
    C = float(max(0.0, el_max + er_max - 10.0))

    bias_rep = np.tile(bias.reshape(1, p.HO), (128, 1)).astype(np.float32)

    # ---- chunk-major table row mapping ----
    # T_full row of node (c, r): k = r // CH -> k*(8*CH) + c*CH + (r % CH)
    ns_idx = np.arange(p.NS, dtype=np.int64)
    base = (ns_idx // p.CH) * (p.NCORES * p.CH) + (ns_idx % p.CH)
    rowlut = np.empty(N, np.int64)
    for c in range(p.NCORES):
        rowlut[c * p.NS : (c + 1) * p.NS] = base + c * p.CH

    # ---- edge prep ----
    order = np.argsort(dst, kind="stable")
    dst_s = dst[order]
    srow_s = rowlut[src[order]]              # table row per edge
    core_bounds = np.searchsorted(dst_s, [c * p.NS for c in range(p.NCORES + 1)])

    per_core = []  # (lo_lists, hi_lists) of (table_row, dst_in_window) per window
    for c in range(p.NCORES):
        e0, e1 = core_bounds[c], core_bounds[c + 1]
        dl = dst_s[e0:e1] - c * p.NS
        sl = srow_s[e0:e1]
        wb = np.searchsorted(dl, [w * p.WIN for w in range(p.NWIN + 1)])
        lo_lists, hi_lists = [], []
        for w in range(p.NWIN):
            a, b = wb[w], wb[w + 1]
            dw = (dl[a:b] - w * p.WIN).astype(np.int64)
            sw = sl[a:b]
            m = sw < p.SPLIT
            lo_lists.append((sw[m], dw[m]))
            hi_lists.append((sw[~m] - p.SPLIT, dw[~m]))
        per_core.append((lo_lists, hi_lists))

    # per-window tile counts: max over cores, >= 1
    TL = [max(1, max(math.ceil(len(pc[0][w][0]) / 128) for pc in per_core))
          for w in range(p.NWIN)]
    TH = [max(1, max(math.ceil(len(pc[1][w][0]) / 128) for pc in per_core))
          for w in range(p.NWIN)]
    cumT = [0]
    for w in range(p.NWIN):
        cumT.append(cumT[-1] + TL[w] + TH[w])
    NT = cumT[-1]                            # total tiles
    NLO = sum(TL)
    NHI = sum(TH)

    F8NP = ml_dtypes.float8_e4m3fn
    eye256 = np.zeros((256, 128), F8NP)
    eye256[:128] = np.eye(128).astype(F8NP)

    in_maps = []
    for c in range(p.NCORES):
        lo_lists, hi_lists = per_core[c]
        dstv = np.full((NT, 128), 255, np.int64)   # 255 = pad (no node)
        lo_cols, hi_cols = [], []
        ci = 0
        while ci < p.NWIN:
            wn = min(p.CHUNK_W, p.NWIN - ci)
            lo_idx, hi_idx = [], []
            for w in range(ci, ci + wn):
                ls, ld = lo_lists[w]
                hs, hd = hi_lists[w]
                li = np.zeros(TL[w] * 128, np.int16)
                li[: len(ls)] = ls.astype(np.int16)
                hi_ = np.zeros(TH[w] * 128, np.int16)
                hi_[: len(hs)] = hs.astype(np.int16)
                lo_idx.append(li)
                hi_idx.append(hi_)
                # dst values into global tiles: lo tiles first, then hi
                t0 = cumT[w]
                dv = dstv[t0 : t0 + TL[w] + TH[w]].reshape(-1)
                dv[: len(ld)] = ld
                dv[TL[w] * 128 : TL[w] * 128 + len(hd)] = hd
            lo_cols.append(_wrap16(np.concatenate(lo_idx)))
            hi_cols.append(_wrap16(np.concatenate(hi_idx)))
            ci += wn
        idx_lo_w = np.concatenate(lo_cols, axis=1)   # [128, NLO*8]
        idx_hi_w = np.concatenate(hi_cols, axis=1)   # [128, NHI*8]

        # host-built one-hot tiles (fp8): oh[t][e, n] = (dst(t,e) == n)
        OH = eye256[dstv]                            # [NT, 128e, 128n] f16
        oh_x = np.ascontiguousarray(
            OH.transpose(1, 0, 2).reshape(128, NT * 128))
        ohT_x = np.ascontiguousarray(
            OH.transpose(2, 0, 1).reshape(128, NT * 128))

        featT = np.zeros((IN, p.NWIN * 128), np.float16)
        featT[:, : p.NS] = feat[c * p.NS : (c + 1) * p.NS].T

        in_maps.append({
            "featT": featT,
            "W_aug": W_aug.astype(np.float16),
            "bias_rep": bias_rep,
            "idx_lo": np.ascontiguousarray(idx_lo_w),
            "idx_hi": np.ascontiguousarray(idx_hi_w),
            "oh": oh_x,
            "ohT": ohT_x,
        })

    meta = dict(TL=tuple(TL), TH=tuple(TH), NT=NT, NLO=NLO, NHI=NHI, C=C)
    return in_maps, meta


def build_nc(p, meta):
    TL, TH = meta["TL"], meta["TH"]
    NT, NLO, NHI, C = meta["NT"], meta["NLO"], meta["NHI"], meta["C"]
    NWIN, NS, ROW = p.NWIN, p.NS, p.ROW
    H, O, HO, IN = p.H, p.O, p.HO, p.IN
    TLMAX2 = max(TL[w] + (TL[w + 1] if w + 1 < NWIN else 0)
                 for w in range(0, NWIN, p.CHUNK_W))
    THMAX2 = max(TH[w] + (TH[w + 1] if w + 1 < NWIN else 0)
                 for w in range(0, NWIN, p.CHUNK_W))
    NTMAX2 = max((TL[w] + TH[w]) + (TL[w + 1] + TH[w + 1] if w + 1 < NWIN else 0)
                 for w in range(0, NWIN, p.CHUNK_W))
    cumT = [0]
    for w in range(NWIN):
        cumT.append(cumT[-1] + TL[w] + TH[w])
    rg = [list(range(p.NCORES))]

    nc = bass.Bass(num_devices=p.NCORES, num_swdge_queues=2)

    featT_x = nc.declare_dram_parameter("featT", [IN, NWIN * 128], F16, isOutput=False)
    waug_x = nc.declare_dram_parameter("W_aug", [IN, p.AUGC], F16, isOutput=False)
    bias_x = nc.declare_dram_parameter("bias_rep", [128, HO], F32, isOutput=False)
    idxlo_x = nc.declare_dram_parameter("idx_lo", [128, NLO * 8], I16, isOutput=False)
    idxhi_x = nc.declare_dram_parameter("idx_hi", [128, NHI * 8], I16, isOutput=False)
    oh_x = nc.declare_dram_parameter("oh", [128, NT * 128], F8, isOutput=False)
    ohT_x = nc.declare_dram_parameter("ohT", [128, NT * 128], F8, isOutput=False)
    out_x = nc.declare_dram_parameter("out", [NS, HO], F32, isOutput=True)

    T_slice = nc.dram_tensor("T_slice", [NS, ROW], F16)
    T_full = nc.dram_tensor("T_full", [p.N, ROW], F16, addr_space="Shared")

    n_lo_rows = min(p.SPLIT, p.N)

    with TileContext(nc) as tc:
        with (
            tc.tile_pool(name="const", bufs=1) as constp,
            tc.tile_pool(name="fT", bufs=3) as ftp,
            tc.tile_pool(name="rowsb", bufs=3) as rowp,
            tc.tile_pool(name="gemmps", bufs=2, space="PSUM") as gpsp,
            tc.tile_pool(name="glo", bufs=3) as glop,
            tc.tile_pool(name="ghi", bufs=3) as ghip,
            tc.tile_pool(name="ohc", bufs=2) as ohcp,
            tc.tile_pool(name="ohtc", bufs=2) as ohtcp,
            tc.tile_pool(name="small", bufs=8) as smp,
            tc.tile_pool(name="rhs", bufs=8) as rhsp,
            tc.tile_pool(name="res", bufs=2) as resp,
            tc.tile_pool(name="accps", bufs=3, space="PSUM") as accp,
            tc.tile_pool(name="erps", bufs=3, space="PSUM") as erpp,
        ):
            nc.gpsimd.load_library(library_config.mlp)

            # ---- resident constants ----
            wa0 = constp.tile([128, p.AUGC], F16)
            wa1 = constp.tile([128, p.AUGC], F16)
            nc.sync.dma_start(out=wa0[:, :], in_=waug_x[0:128, :])
            nc.sync.dma_start(out=wa1[:, :], in_=waug_x[128:256, :])
            bias_sb = constp.tile([128, HO], F32)
            nc.sync.dma_start(out=bias_sb[:, :], in_=bias_x[:, :])
            idxlo_sb = constp.tile([128, NLO * 8], I16)
            nc.sync.dma_start(out=idxlo_sb[:, :], in_=idxlo_x[:, :])
            idxhi_sb = constp.tile([128, NHI * 8], I16)
            nc.sync.dma_start(out=idxhi_sb[:, :], in_=idxhi_x[:, :])
            erall = constp.tile([128, NWIN * H], F16)

            # ---- GEMM phase: project node slice, build table rows ----
            # AllGather chunk k fires right after the window completing its rows.
            ag_after = {((k + 1) * p.CH - 1) // 128: k for k in range(p.NCH)}
            for nt in range(NWIN):
                rows = min(128, NS - nt * 128)
                fT0 = ftp.tile([128, 128], F16, tag="fT0")
                fT1 = ftp.tile([128, 128], F16, tag="fT1")
                nc.scalar.dma_start(out=fT0[:, :], in_=featT_x[0:128, nt * 128 : (nt + 1) * 128])
                nc.scalar.dma_start(out=fT1[:, :], in_=featT_x[128:256, nt * 128 : (nt + 1) * 128])
                ps = gpsp.tile([128, 512], F32, tag="gps", name="gps_t")[:, : p.AUGC]
                nc.tensor.matmul(ps[:, :], lhsT=fT0[:, :],
                                 rhs=wa0[:, :], start=True, stop=False)
                nc.tensor.matmul(ps[:, :], lhsT=fT1[:, :],
                                 rhs=wa1[:, :], start=False, stop=True)
                row_sb = rowp.tile([128, ROW], F16)
                # ft -> f16
                nc.vector.tensor_copy(out=row_sb[:, 0:HO], in_=ps[:, 0:HO])
                # er -> resident SBUF (erall)
                nc.vector.tensor_copy(out=erall[:, nt * H : (nt + 1) * H],
                                      in_=ps[:, HO + H : HO + 2 * H])
                # el -> f32 at COL_EL (bitcast view)
                nc.vector.tensor_copy(out=row_sb[:, p.COL_EL : p.COL_EL + 16].bitcast(F32),
                                      in_=ps[:, HO : HO + H])
                nc.sync.dma_start(out=T_slice[nt * 128 : nt * 128 + rows, :],
                                  in_=row_sb[:rows, :])
                k = ag_after.get(nt)
                if k is not None:
                    nc.gpsimd.collective_compute(
                        "AllGather", ALU.bypass, replica_groups=rg,
                        ins=[T_slice[k * p.CH : (k + 1) * p.CH, :]],
                        outs=[T_full[k * p.NCORES * p.CH : (k + 1) * p.NCORES * p.CH, :]],
                    )

            # ---- edge phase ----
            _regs = {}

            def nreg(v):
                if v not in _regs:
                    _regs[v] = nc.gpsimd.to_reg(v)
                return _regs[v]

            lo_col = 0
            hi_col = 0
            ci = 0
            while ci < NWIN:
                wn = min(p.CHUNK_W, NWIN - ci)
                nlo = sum(TL[ci : ci + wn])
                nhi = sum(TH[ci : ci + wn])
                nt_ch = nlo + nhi
                g_lo = glop.tile([128, TLMAX2, ROW], F16, tag="glo", name="glo_t")[:, :nlo, :]
                g_hi = ghip.tile([128, THMAX2, ROW], F16, tag="ghi", name="ghi_t")[:, :nhi, :]
                nc.gpsimd.dma_gather(
                    out_ap=g_lo[:, :, :], in_ap=T_full[0:n_lo_rows, :],
                    idxs_ap=idxlo_sb[:, lo_col : lo_col + nlo * 8],
                    num_idxs=nlo * 128, num_idxs_reg=nreg(nlo * 128), elem_size=ROW,
                    single_packet=False, queue_num=0)
                nc.gpsimd.dma_gather(
                    out_ap=g_hi[:, :, :], in_ap=T_full[p.SPLIT : p.N, :],
                    idxs_ap=idxhi_sb[:, hi_col : hi_col + nhi * 8],
                    num_idxs=nhi * 128, num_idxs_reg=nreg(nhi * 128), elem_size=ROW,
                    single_packet=False, queue_num=1)
                lo_col += nlo * 8
                hi_col += nhi * 8
                oh_ch = ohcp.tile([128, NTMAX2, 128], F8, tag="ohc", name="ohc_t")[:, :nt_ch, :]
                ohT_ch = ohtcp.tile([128, NTMAX2, 128], F8, tag="ohtc", name="ohtc_t")[:, :nt_ch, :]
                nc.scalar.dma_start(
                    out=oh_ch[:, :, :],
                    in_=oh_x[:, cumT[ci] * 128 : cumT[ci + wn] * 128])
                nc.sync.dma_start(
                    out=ohT_ch[:, :, :],
                    in_=ohT_x[:, cumT[ci] * 128 : cumT[ci + wn] * 128])

                for wi in range(wn):
                    w = ci + wi
                    rows = min(128, NS - w * 128)
                    acc = accp.tile([128, 512], F32, tag="acc", name="acc_t")[:, : HO + H]
                    ntw = TL[w] + TH[w]
                    for t in range(ntw):
                        lo = t < TL[w]
                        if lo:
                            j = (TL[ci] if wi else 0) + t
                            grow = g_lo[:, j, :]
                        else:
                            j = (TH[ci] if wi else 0) + (t - TL[w])
                            grow = g_hi[:, j, :]
                        jc = cumT[w] - cumT[ci] + t
                        oh = oh_ch[:, jc, :]
                        ohT = ohT_ch[:, jc, :]
                        er_ps = erpp.tile([128, 512], F32, tag="erps", name="erps_t")[:, :H]
                        nc.tensor.matmul(er_ps[:, :], lhsT=ohT,
                                         rhs=erall[:, w * H : (w + 1) * H],
                                         start=True, stop=True)
                        logit = smp.tile([128, H], F32, tag="logit")
                        nc.vector.tensor_tensor(
                            out=logit[:, :],
                            in0=grow[:, p.COL_EL : p.COL_EL + 16].bitcast(F32),
                            in1=er_ps[:, :], op=ALU.add)
                        logit2 = smp.tile([128, H], F32, tag="logit2")
                        nc.vector.scalar_tensor_tensor(
                            out=logit2[:, :], in0=logit[:, :], scalar=0.2,
                            in1=logit[:, :], op0=ALU.mult, op1=ALU.max)
                        rhs_t = rhsp.tile([128, HO + H], F16)
                        nc.scalar.activation(out=rhs_t[:, HO : HO + H],
                                             in_=logit2[:, :], func=ACTF.Exp,
                                             bias=-C, scale=1.0)
                        nc.vector.tensor_tensor(
                            out=rhs_t[:, 0:HO].rearrange("p (h o) -> p h o", h=H),
                            in0=grow[:, 0:HO].rearrange("p (h o) -> p h o", h=H),
                            in1=rhs_t[:, HO : HO + H].unsqueeze(-1).broadcast_to([128, H, O]),
                            op=ALU.mult)
                        nc.tensor.matmul(acc[:, :], lhsT=oh, rhs=rhs_t[:, :],
                                         start=(t == 0), stop=(t == ntw - 1))
                    # window epilogue
                    den = smp.tile([128, H], F32, tag="den")
                    nc.vector.tensor_scalar(out=den[:, :], in0=acc[:, HO : HO + H],
                                            scalar1=1e-30, scalar2=None, op0=ALU.max)
                    rec = smp.tile([128, H], F32, tag="rec")
                    nc.vector.reciprocal(out=rec[:, :], in_=den[:, :])
                    res = resp.tile([128, HO], F32, tag="res")
                    nc.vector.tensor_tensor(
                        out=res[:, :].rearrange("p (h o) -> p h o", h=H),
                        in0=acc[:, 0:HO].rearrange("p (h o) -> p h o", h=H),
                        in1=rec[:, :].unsqueeze(-1).broadcast_to([128, H, O]),
                        op=ALU.mult)
                    res2 = resp.tile([128, HO], F32, tag="res2")
                    nc.vector.tensor_tensor(out=res2[:, :], in0=res[:, :],
                                            in1=bias_sb[:, :], op=ALU.add)
                    nc.sync.dma_start(out=out_x[w * 128 : w * 128 + rows, :],
                                      in_=res2[:rows, :])
                ci += wn
    from concourse.library_overlay import lower_extended_insts

    lower_extended_insts(nc)
    _split_multi_waits(nc)
    return nc


_CACHE = {}


def kernel(feat, W, attn_l, attn_r, bias, src, dst):
    p = Params()
    in_maps, meta = host_prep(p, feat, W, attn_l, attn_r, bias, src, dst)
    key = (meta["TL"], meta["TH"], round(meta["C"], 6))
    if key not in _CACHE:
        _CACHE[key] = build_nc(p, meta)
    nc = _CACHE[key]
    res = run_bass_kernel_spmd(
        nc, in_maps, list(range(p.NCORES)),
        trace=bool(os.environ.get("BASS_TRACE")),
    )
    global LAST_EXEC_NS
    LAST_EXEC_NS = res.exec_time_ns
    out = np.concatenate([res.results[c]["out"] for c in range(p.NCORES)], axis=0)
    return out.reshape(p.N, p.H, p.O).astype(np.float32)


LAST_EXEC_NS = None
